# revision 22
# baseline (speedup 1.0000x reference)
"""Trainium2 Bass kernel for the CfC (closed-form continuous-time) RNN.

Model (reference semantics):
    per step t:  z = [x_t, h]                       [B, F+H]
                 z1 = 1.7159*tanh(0.666*(z@W1+b1))  [B, U]
                 z2 = 1.7159*tanh(0.666*(z1@W2+b2)) [B, U]
                 ff1 = tanh(z2@Wff1+bff1); ff2 = tanh(z2@Wff2+bff2)
                 ti  = sigmoid((z2@Wta+bta)*t + (z2@Wtb+btb))
                 h   = ff1 + ti*(ff2-ff1)
    out = hs @ Wout + bout                          [B, T, OUT]

Sharding: data-parallel over batch B=512 across 8 cores (64 rows each);
the sequential time scan stays local per core, weights replicated.

On-chip layout: activations are kept feature-major ("fm"): SBUF tiles of
[128 partitions = feature chunk, batch=64 columns], chunks packed side by
side.  All matmuls keep weights stationary (lhsT=[K,M] feature chunks)
and stream the 64 batch columns; PSUM regions pack M-chunks at 64-column
granularity.  Biases are injected with small K<=6 "opener" matmuls that
also serve as the bank-clearing (start=True) accumulation opener.
"""

import os
import sys
import time

import numpy as np

for _p in ("/opt/trn_rl_repo",):
    if os.path.isdir(_p) and _p not in sys.path:
        sys.path.append(_p)

import ml_dtypes  # noqa: E402
import bass_rust as _bass_rust  # noqa: E402
import concourse.bass as bass  # noqa: E402
import concourse.tile as tile  # noqa: E402
from concourse import mybir  # noqa: E402
from concourse.bass_utils import run_bass_kernel_spmd  # noqa: E402
from concourse.vector_clock import ScopedClock  # noqa: E402
from contextlib import ExitStack  # noqa: E402

# ---------------------------------------------------------------------------
# The walrus build in this container rejects >1-2 sem waits on a single
# TPB_CTRL (drain) instruction; Tile's exit path puts all final-clock waits
# on one drain.  Split them across individual nops instead.
_ORIG_DRAIN = tile.TileContext._drain_and_barrier


def _drain_and_barrier_split(self, tick_clock, wait_clock):
    nc = self.nc
    drain_inst = nc.sync.drain()
    wait_clock.add_sem_waits(
        drain_inst.ins, ScopedClock({None: tick_clock.global_clock})
    )
    si = drain_inst.ins.sync_info
    waits = list(si.on_wait) if si and si.on_wait else []
    if len(waits) > 1:
        si.on_wait.clear()
        by_key = {}
        for h in self.sems.allocated().values():
            by_key[getattr(h, "id", None)] = h
            by_key[getattr(h, "name", None)] = h
        for w in waits:
            h = by_key.get(w.id) or by_key.get(w.ant_name)
            assert h is not None, (w.id, w.ant_name)
            nc.sync.nop(nofuse=True).wait_op(h, w.wait_value, "sem-ge")
    nc.all_engine_barrier()
    assert self.sems is not None
    popped = nc._tile_sem_poison_stack.pop()
    assert popped is self._sem_poison
    nc.clear_and_free_semaphores(list(self.sems.allocated().values()))
    nc.all_engine_barrier()


tile.TileContext._drain_and_barrier = _drain_and_barrier_split

# The same walrus limit applies to every instruction: at most _WAIT_CAP sem
# waits can ride on one instruction.  Wrap Tile's wait-assignment pass to
# move excess waits onto injected same-engine nops placed just before the
# affected instruction (engine program order keeps the gating exact).
_WAIT_CAP = 1
_RustTileClockWait = tile.TileClockWait


class _CappedTileClockWait:
    def __init__(self, tc, ordered, *a, **k):
        self._inner = _RustTileClockWait(tc, ordered, *a, **k)
        self._tc = tc
        self._ordered = ordered

    def __getattr__(self, name):
        return getattr(self._inner, name)

    def assign_waits(self, bb_name):
        self._inner.assign_waits(bb_name)
        nc = self._tc.nc
        for insts in self._ordered.values():
            out = []
            for inst in insts:
                si = inst.sync_info
                waits = list(si.on_wait) if si and si.on_wait else []
                if len(waits) > _WAIT_CAP:
                    extras, keep = waits[:-_WAIT_CAP], waits[-_WAIT_CAP:]
                    si.on_wait.clear()
                    for w in keep:
                        si.on_wait.append(w)
                    for i in range(0, len(extras), _WAIT_CAP):
                        nop = mybir.InstNoOp(
                            name=nc.get_next_instruction_name(), ins=[], outs=[]
                        )
                        nop.engine = inst.engine
                        nop.bass_nofuse = True
                        nop.sync_info = _bass_rust.SyncInfo(
                            on_wait=list(extras[i : i + _WAIT_CAP]), on_update=[]
                        )
                        out.append(nop)
                out.append(inst)
            insts[:] = out


tile.TileClockWait = _CappedTileClockWait

# ---------------------------------------------------------------------------
# Problem dimensions (hardcoded per spec).
B, T, F, H, U, OUT = 512, 256, 128, 256, 512, 128
T_MAX = 256.0
NCORES = 8
J = B // NCORES  # 64 batch rows per core
LECUN_A, LECUN_B = 1.7159, 0.666
NW = 60  # weight tiles: 12 (W1) + 16 (W2) + 32 (heads)
WIN = 32  # steps per DMA window for the per-step panels

F32 = mybir.dt.float32
BF16 = mybir.dt.bfloat16


def _build_nc(use_bf16: bool, t_steps: int = T, zero_bias: bool = False):
    """Build the single-core Bass module (SPMD: same program on all cores).

    zero_bias: all bias vectors are known-zero (true for this problem's
    setup_inputs), so the bias-injection matmuls are dropped and the
    bank-clearing start=True role moves to the first weight matmul of each
    PSUM bank, with explicit ordering deps to the other region openers.
    """
    DT = BF16 if use_bf16 else F32
    nc = bass.Bass("TRN2", target_bir_lowering=False, debug=False)

    d_x = nc.declare_dram_parameter("x_fm", [128, t_steps * J], DT, isOutput=False)
    d_wb = nc.declare_dram_parameter("wblob", [128, NW * 128], DT, isOutput=False)
    d_wo = nc.declare_dram_parameter("woutb", [128, 2 * OUT], BF16, isOutput=False)
    d_bo = nc.declare_dram_parameter("boutrow", [1, OUT], BF16, isOutput=False)
    d_on = nc.declare_dram_parameter("onesrow", [1, 128], BF16, isOutput=False)
    d_b1 = nc.declare_dram_parameter("B1", [4, 128], DT, isOutput=False)
    d_b2 = nc.declare_dram_parameter("B2", [4, 128], DT, isOutput=False)
    d_bh = nc.declare_dram_parameter("Bh", [6, 128], DT, isOutput=False)
    d_bt = nc.declare_dram_parameter("Bta", [2, 128], DT, isOutput=False)
    d_k4 = nc.declare_dram_parameter("blk4", [4, 4 * J], DT, isOutput=False)
    d_qc = nc.declare_dram_parameter("Qc", [6, 8 * J], DT, isOutput=False)
    d_tp = nc.declare_dram_parameter("tpair", [t_steps, 2, 128], DT, isOutput=False)
    d_tb = nc.declare_dram_parameter("t2rep", [t_steps, 128], F32, isOutput=False)
    d_y = nc.declare_dram_parameter("y", [J, t_steps, OUT], F32, isOutput=True)

    Tanh = mybir.ActivationFunctionType.Tanh
    Sigm = mybir.ActivationFunctionType.Sigmoid

    win = min(WIN, t_steps)
    with tile.TileContext(nc) as tc, ExitStack() as ctx:
        res = ctx.enter_context(tc.tile_pool(name="res", bufs=1))
        ptb = ctx.enter_context(tc.tile_pool(name="ptb", bufs=2))
        ptp = ctx.enter_context(tc.tile_pool(name="ptp", bufs=2))
        pwork = ctx.enter_context(tc.tile_pool(name="pwork", bufs=3))
        pstage = ctx.enter_context(tc.tile_pool(name="pstage", bufs=4))
        phring = ctx.enter_context(tc.tile_pool(name="phring", bufs=2))
        pz1 = ctx.enter_context(tc.tile_pool(name="pz1", bufs=1, space="PSUM"))
        pz2 = ctx.enter_context(tc.tile_pool(name="pz2", bufs=1, space="PSUM"))
        pgate = ctx.enter_context(tc.tile_pool(name="pgate", bufs=1, space="PSUM"))
        pff0 = ctx.enter_context(tc.tile_pool(name="pff0", bufs=1, space="PSUM"))
        pff1 = ctx.enter_context(tc.tile_pool(name="pff1", bufs=1, space="PSUM"))
        pop = ctx.enter_context(tc.tile_pool(name="pop", bufs=2, space="PSUM"))

        # --- resident tiles -------------------------------------------------
        x_sb = res.tile([128, t_steps * J], DT, tag="x_sb")
        nc.sync.dma_start(x_sb[:], d_x.ap()[:])
        wb = res.tile([128, NW * 128], DT, tag="wb")
        nc.sync.dma_start(wb[:], d_wb.ap()[:])
        wo = res.tile([128, 2 * OUT], BF16, tag="wo")
        nc.sync.dma_start(wo[:], d_wo.ap()[:])
        if not zero_bias:
            bo = res.tile([1, OUT], BF16, tag="bo")
            nc.sync.dma_start(bo[:], d_bo.ap()[:])
            ones = res.tile([1, 128], BF16, tag="ones")
            nc.sync.dma_start(ones[:], d_on.ap()[:])
            b1 = res.tile([4, 128], DT, tag="b1")
            nc.sync.dma_start(b1[:], d_b1.ap()[:])
            b2 = res.tile([4, 128], DT, tag="b2")
            nc.sync.dma_start(b2[:], d_b2.ap()[:])
            bg = res.tile([2, 128], DT, tag="bg")
            nc.sync.dma_start(bg[:], d_bh.ap()[0:2])
            bf0 = res.tile([2, 128], DT, tag="bf0")
            nc.sync.dma_start(bf0[:], d_bh.ap()[2:4])
            bf1 = res.tile([2, 128], DT, tag="bf1")
            nc.sync.dma_start(bf1[:], d_bh.ap()[4:6])
            bta = res.tile([2, 128], DT, tag="bta")
            nc.sync.dma_start(bta[:], d_bt.ap()[:])
            blk4 = res.tile([4, 4 * J], DT, tag="blk4")
            nc.sync.dma_start(blk4[:], d_k4.ap()[:])
            qg = res.tile([2, 4 * J], DT, tag="qg")
            nc.sync.dma_start(qg[:], d_qc.ap()[0:2, 0 : 4 * J])
            qf0 = res.tile([2, 2 * J], DT, tag="qf0")
            nc.sync.dma_start(qf0[:], d_qc.ap()[2:4, 0 : 2 * J])
            qf1 = res.tile([2, 2 * J], DT, tag="qf1")
            nc.sync.dma_start(qf1[:], d_qc.ap()[4:6, 0 : 2 * J])
        # hs0/hs1: full history of h (bf16), chunk-major: hs_c[:, t*J:(t+1)*J]
        # holds h features [c*128, (c+1)*128) at step t
        hs0 = res.tile([128, t_steps * J], BF16, tag="hs0")
        hs1 = res.tile([128, t_steps * J], BF16, tag="hs1")
        hz = res.tile([128, 128], DT, tag="hz")
        nc.gpsimd.memset(hz[:], 0.0)

        def wt(i):
            return wb[:, i * 128 : (i + 1) * 128]

        y_v = d_y.ap().rearrange("b t o -> t b o")

        tbw = tpw = None
        hprev_f32 = None  # fp32 variant: previous-step h ring tile

        for t in range(t_steps):
            w = t % win
            if w == 0:
                # per-step panels for this window of steps
                tbw = ptb.tile([128, win * 128], F32, tag="tbw")
                nc.gpsimd.dma_start(
                    tbw[:],
                    d_tb.ap()[t : t + win].flatten().partition_broadcast(128),
                )
                if not zero_bias:
                    tpw = ptp.tile([2, win * 128], DT, tag="tpw")
                    nc.sync.dma_start(
                        tpw[:].rearrange("r (t c) -> r t c", t=win),
                        d_tp.ap()[t : t + win].rearrange("t r c -> r t c"),
                    )
            tb_sl = tbw[:, w * 128 : (w + 1) * 128]
            tp_sl = None if zero_bias else tpw[:, w * 128 : (w + 1) * 128]

            if t == 0:
                hp0, hp1 = hz[:, 0:J], hz[:, J : 2 * J]
            elif use_bf16:
                hp0 = hs0[:, (t - 1) * J : t * J]
                hp1 = hs1[:, (t - 1) * J : t * J]
            else:
                hp0 = hprev_f32[:, 0:J]
                hp1 = hprev_f32[:, J : 2 * J]
            zrhs = (x_sb[:, t * J : (t + 1) * J], hp0, hp1)

            # ---- z1 = tanh(0.666*(z@W1 + b1)) -----------------------------
            z1p = pz1.tile([128, 4 * J], F32, tag="z1p")
            first = None
            if not zero_bias:
                first = nc.tensor.matmul(
                    z1p[:], b1[:], blk4[:], start=True, stop=False
                )
            for m in range(4):
                for k in range(3):
                    mi = nc.tensor.matmul(
                        z1p[:, m * J : (m + 1) * J],
                        wt(k * 4 + m),
                        zrhs[k],
                        start=(zero_bias and m == 0 and k == 0),
                        stop=(m == 3 and k == 2),
                    )
                    if first is None:
                        first = mi
                    elif zero_bias and k == 0:
                        tile.add_dep_helper(
                            mi.ins, first.ins, sync=False, reason="bank clear order"
                        )
            z1s = pwork.tile([128, 4 * J], DT, tag="z1s")
            nc.scalar.activation(z1s[:, 0 : 2 * J], z1p[:, 0 : 2 * J], Tanh, scale=LECUN_B)
            nc.scalar.activation(z1s[:, 2 * J : 4 * J], z1p[:, 2 * J : 4 * J], Tanh, scale=LECUN_B)

            # ---- z2 = tanh(0.666*(z1@(a*W2) + b2)) ------------------------
            z2p = pz2.tile([128, 4 * J], F32, tag="z2p")
            first = None
            if not zero_bias:
                first = nc.tensor.matmul(
                    z2p[:], b2[:], blk4[:], start=True, stop=False
                )
            for m in range(4):
                for k in range(4):
                    mi = nc.tensor.matmul(
                        z2p[:, m * J : (m + 1) * J],
                        wt(12 + k * 4 + m),
                        z1s[:, k * J : (k + 1) * J],
                        start=(zero_bias and m == 0 and k == 0),
                        stop=(m == 3 and k == 3),
                    )
                    if first is None:
                        first = mi
                    elif zero_bias and k == 0:
                        tile.add_dep_helper(
                            mi.ins, first.ins, sync=False, reason="bank clear order"
                        )
            z2s = pwork.tile([128, 4 * J], DT, tag="z2s")
            nc.scalar.activation(z2s[:, 0 : 2 * J], z2p[:, 0 : 2 * J], Tanh, scale=LECUN_B)
            nc.scalar.activation(z2s[:, 2 * J : 4 * J], z2p[:, 2 * J : 4 * J], Tanh, scale=LECUN_B)

            # ---- heads, three psum banks so reads never overlap writes:
            #   gate bank: [ta_c0 | ta_c1 | tb_c0 | tb_c1]
            #   ff bank c: [ff1_c | ff2_c]          (one per feature chunk)
            hdg = pgate.tile([128, 4 * J], F32, tag="hdg")
            hdf0 = pff0.tile([128, 2 * J], F32, tag="hdf0")
            hdf1 = pff1.tile([128, 2 * J], F32, tag="hdf1")
            hdf = (hdf0, hdf1)
            firsts = [None, None, None]  # gate, ff0, ff1
            if not zero_bias:
                firsts[0] = nc.tensor.matmul(hdg[:], bg[:], qg[:], start=True, stop=False)
                nc.tensor.matmul(
                    hdg[:, 2 * J : 4 * J], bta[:], tp_sl, start=False, stop=False
                )
                firsts[1] = nc.tensor.matmul(hdf[0][:], bf0[:], qf0[:], start=True, stop=False)
                firsts[2] = nc.tensor.matmul(hdf[1][:], bf1[:], qf1[:], start=True, stop=False)

            def head_mm(s8, k):
                # s8: global head slot (wblob order); banks: 0-3 gate, 4-5 ff0, 6-7 ff1
                bi, sl = (0, s8) if s8 < 4 else (1 + (s8 - 4) // 2, s8 % 2)
                bank = hdg if bi == 0 else hdf[bi - 1]
                last_sl = 3 if bi == 0 else 1
                mi = nc.tensor.matmul(
                    bank[:, sl * J : (sl + 1) * J],
                    wt(28 + k * 8 + s8),
                    z2s[:, k * J : (k + 1) * J],
                    start=(zero_bias and sl == 0 and k == 0),
                    stop=(sl == last_sl and k == 3),
                )
                if firsts[bi] is None:
                    firsts[bi] = mi
                elif zero_bias and k == 0 and sl != 0:
                    tile.add_dep_helper(
                        mi.ins, firsts[bi].ins, sync=False, reason="bank clear order"
                    )

            for s8 in range(4):  # ta/tb slots first -> early sigmoid
                for k in range(4):
                    head_mm(s8, k)
            u_s = pwork.tile([128, 128], F32, tag="u_s")
            nc.vector.tensor_mul(u_s[:], hdg[:, 0 : 2 * J], tb_sl)
            s_s = pwork.tile([128, 128], F32, tag="s_s")
            nc.vector.tensor_add(s_s[:], u_s[:], hdg[:, 2 * J : 4 * J])
            tis = pwork.tile([128, 128], DT, tag="tis")
            nc.scalar.activation(tis[:], s_s[:], Sigm)

            if not use_bf16:
                hcur = phring.tile([128, 128], F32, tag="hcur")
            for c in range(2):
                for k in range(4):
                    head_mm(4 + 2 * c, k)
                for k in range(4):
                    head_mm(5 + 2 * c, k)
                ffc = pwork.tile([128, 2 * J], DT, tag=f"ffc{c}")
                nc.scalar.activation(ffc[:], hdf[c][:], Tanh)
                d_s = pwork.tile([128, J], DT, tag=f"d_s{c}")
                nc.vector.tensor_sub(d_s[:], ffc[:, J : 2 * J], ffc[:, 0:J])
                p_s = pwork.tile([128, J], DT, tag=f"p_s{c}")
                nc.vector.tensor_mul(p_s[:], tis[:, c * J : (c + 1) * J], d_s[:])
                hs_c = hs0 if c == 0 else hs1
                if use_bf16:
                    nc.vector.tensor_add(
                        hs_c[:, t * J : (t + 1) * J], ffc[:, 0:J], p_s[:]
                    )
                else:
                    nc.vector.tensor_add(
                        hcur[:, c * J : (c + 1) * J], ffc[:, 0:J], p_s[:]
                    )
                    nc.scalar.copy(
                        hs_c[:, t * J : (t + 1) * J],
                        hcur[:, c * J : (c + 1) * J],
                    )
            if not use_bf16:
                hprev_f32 = hcur

            # ---- interleaved output projection: out = hs@Wout + bout ------
            if t % 2 == 1:
                i = (t - 1) // 2
                opp = pop.tile([128, OUT], F32, tag="opp")
                if not zero_bias:
                    nc.tensor.matmul(opp[:], ones[:], bo[:], start=True, stop=False)
                for c, hs_c in enumerate((hs0, hs1)):
                    nc.tensor.matmul(
                        opp[:],
                        hs_c[:, 2 * i * J : (2 * i + 2) * J],
                        wo[:, c * OUT : (c + 1) * OUT],
                        start=(zero_bias and c == 0),
                        stop=(c == 1),
                    )
                stg = pstage.tile([128, OUT], F32, tag="stg")
                if i % 2 == 0:
                    nc.scalar.copy(stg[:], opp[:])
                else:
                    nc.vector.tensor_copy(stg[:], opp[:])
                nc.sync.dma_start(y_v[2 * i : 2 * i + 2], stg[:])

    return nc


def _prep_host(inputs, use_bf16: bool, t_steps: int = T):
    """Build the per-core input maps from the full-size problem inputs."""
    DTnp = ml_dtypes.bfloat16 if use_bf16 else np.float32

    def dt(a):
        return np.ascontiguousarray(np.asarray(a, dtype=np.float32)).astype(DTnp)

    x = np.asarray(inputs["x"], np.float32)[:, :t_steps]
    ts = np.asarray(inputs["ts"], np.float32)[:, :t_steps]
    W1 = np.asarray(inputs["W1"], np.float32)
    W2 = np.asarray(inputs["W2"], np.float32) * LECUN_A
    WH = (
        np.concatenate(
            [
                np.asarray(inputs["Wff1"], np.float32),
                np.asarray(inputs["Wff2"], np.float32),
                np.asarray(inputs["Wta"], np.float32),
                np.asarray(inputs["Wtb"], np.float32),
            ],
            axis=1,
        )
        * LECUN_A
    )

    tiles = []
    for k in range(3):
        for m in range(4):
            tiles.append(W1[k * 128 : (k + 1) * 128, m * 128 : (m + 1) * 128])
    for k in range(4):
        for m in range(4):
            tiles.append(W2[k * 128 : (k + 1) * 128, m * 128 : (m + 1) * 128])
    # head slot order: [ta_c0, ta_c1, tb_c0, tb_c1, ff1_c0, ff2_c0, ff1_c1, ff2_c1]
    # WH columns: ff1 0:256, ff2 256:512, ta 512:768, tb 768:1024
    slotcol = (512, 640, 768, 896, 0, 256, 128, 384)
    for k in range(4):
        for s8 in range(8):
            c0 = slotcol[s8]
            tiles.append(WH[k * 128 : (k + 1) * 128, c0 : c0 + 128])
    wblob = dt(np.concatenate(tiles, axis=1))

    Wout = np.asarray(inputs["Wout"], np.float32)
    woutb = np.concatenate([Wout[0:128], Wout[128:256]], axis=1).astype(
        ml_dtypes.bfloat16
    )
    boutrow = np.asarray(inputs["bout"], np.float32)[None, :].astype(ml_dtypes.bfloat16)
    onesrow = np.ones((1, 128), ml_dtypes.bfloat16)

    def chunks(v, n):
        return np.stack([v[i * 128 : (i + 1) * 128] for i in range(n)])

    B1 = dt(chunks(np.asarray(inputs["b1"], np.float32), 4))
    B2 = dt(chunks(np.asarray(inputs["b2"], np.float32), 4))
    bff1 = chunks(np.asarray(inputs["bff1"], np.float32), 2)
    bff2 = chunks(np.asarray(inputs["bff2"], np.float32), 2)
    btb = chunks(np.asarray(inputs["btb"], np.float32), 2)
    # Bh rows target head-psum slots (2..7): [tb_c0, tb_c1, ff1_c0, ff2_c0, ff1_c1, ff2_c1]
    Bh = dt(
        np.stack([btb[0], btb[1], bff1[0], bff2[0], bff1[1], bff2[1]], axis=0)
    )
    Bta = dt(chunks(np.asarray(inputs["bta"], np.float32), 2))

    blk4 = np.zeros((4, 4 * J), np.float32)
    for i in range(4):
        blk4[i, i * J : (i + 1) * J] = 1.0
    blk4 = dt(blk4)
    qcm = np.zeros((6, 8 * J), np.float32)
    qcm[0, 2 * J : 3 * J] = 1.0  # btb_c0 -> gate bank block 2
    qcm[1, 3 * J : 4 * J] = 1.0  # btb_c1 -> gate bank block 3
    for j in range(4):  # ff banks: rows (2,3)->bank c0 blocks 0,1; (4,5)->bank c1
        qcm[2 + j, (j % 2) * J : (j % 2 + 1) * J] = 1.0
    qcm = dt(qcm)

    common = dict(
        wblob=wblob, woutb=woutb, boutrow=boutrow, onesrow=onesrow,
        B1=B1, B2=B2, Bh=Bh, Bta=Bta, blk4=blk4, Qc=qcm,
    )

    in_maps = []
    for c in range(NCORES):
        xs = x[c * J : (c + 1) * J]  # [J, t, F]
        x_fm = dt(xs.transpose(2, 1, 0).reshape(128, t_steps * J))
        tv = (ts[c * J : (c + 1) * J] / T_MAX).T  # [t, J]
        t2rep = np.concatenate([tv, tv], axis=1).astype(np.float32)  # [t, 128]
        tpair = np.zeros((t_steps, 2, 128), np.float32)
        tpair[:, 0, 0:J] = tv
        tpair[:, 1, J : 2 * J] = tv
        in_maps.append(dict(common, x_fm=x_fm, t2rep=t2rep, tpair=dt(tpair)))
    return in_maps


_CACHE = {}


def _get_nc(use_bf16: bool, t_steps: int = T, zero_bias: bool = False):
    key = (use_bf16, t_steps, zero_bias)
    if key not in _CACHE:
        _CACHE[key] = _build_nc(use_bf16, t_steps, zero_bias)
    return _CACHE[key]


USE_BF16 = True
_BIAS_NAMES = ("b1", "b2", "bff1", "bff2", "bta", "btb", "bout")


def kernel(**inputs) -> np.ndarray:
    zb = all(not np.any(np.asarray(inputs[n])) for n in _BIAS_NAMES)
    nc = _get_nc(USE_BF16, T, zb)
    in_maps = _prep_host(inputs, USE_BF16)
    res = run_bass_kernel_spmd(nc, in_maps, list(range(NCORES)))
    out = np.concatenate([res.results[c]["y"] for c in range(NCORES)], axis=0)
    return np.ascontiguousarray(out.astype(np.float32))


if __name__ == "__main__":
    rng = np.random.default_rng(0)
    dummy = {
        "x": rng.normal(size=(B, T, F)).astype(np.float32),
        "ts": rng.uniform(1.0, T_MAX, size=(B, T)).astype(np.float32),
        "W1": rng.normal(size=(F + H, U)).astype(np.float32) / 19.6,
        "b1": np.zeros(U, np.float32),
        "W2": rng.normal(size=(U, U)).astype(np.float32) / 22.6,
        "b2": np.zeros(U, np.float32),
        "Wff1": rng.normal(size=(U, H)).astype(np.float32) / 22.6,
        "bff1": np.zeros(H, np.float32),
        "Wff2": rng.normal(size=(U, H)).astype(np.float32) / 22.6,
        "bff2": np.zeros(H, np.float32),
        "Wta": rng.normal(size=(U, H)).astype(np.float32) / 22.6,
        "bta": np.zeros(H, np.float32),
        "Wtb": rng.normal(size=(U, H)).astype(np.float32) / 22.6,
        "btb": np.zeros(H, np.float32),
        "Wout": rng.normal(size=(H, OUT)).astype(np.float32) / 16.0,
        "bout": np.zeros(OUT, np.float32),
    }
    t0 = time.time()
    y = kernel(**dummy)
    print("kernel done", y.shape, time.time() - t0)
